# revision 5
# baseline (speedup 1.0000x reference)
"""Multi-head attention (B=4, S=2048, D=1024, H=16, Dh=64) on 8 trn2 cores.

Sharding: core c handles batch b=c//2 and query-half qh=c%2 (1024 query rows,
all 2048 keys). No cross-core communication; K/V projections are duplicated
across the two cores of a batch (cheap vs. the 1 GiB attn output write).

Per-core dataflow (everything "transposed" orientation to avoid on-chip
transposes):
  phase 1: Q^T = Wq^T.T @ xq^T        [H*Dh, 1024]   (SBUF resident)
           K^T all heads              [H*Dh, 2048]   (spilled to DRAM scratch)
           V all heads + ones column  [2048, H*(64+1)] (spilled, gap layout)
  phase 2 (per head): S^T = K_h Q_h^T  [2048k, 1024q]  (f32r matmuls)
           P^T = exp(S^T/8)           (ACT, PSUM->SBUF staging)
           ctx^T[65,1024] = [V_h|1].T @ P^T   (row 64 = softmax row-sums)
           rinv = 1/sums; broadcast via ones-matmul; normalize P^T in place
           (DVE) -> DMA attn^T to HBM; normalize ctx rows 0:64 into ctx_sb
  phase 3: fc = ctx^T.T @ Wfc^T (natural [q, D]) + residual, LayerNorm, out.

attn is returned transposed per (head): host transposes during unshard.
"""

import numpy as np

import concourse.bass as bass  # noqa: F401  (bass types used via tile/bacc)
import concourse.tile as tile
from concourse import bacc, mybir
from concourse.bass_utils import run_bass_kernel_spmd

F32 = mybir.dt.float32
F32R = mybir.dt.float32r
AF = mybir.ActivationFunctionType
MULT = mybir.AluOpType.mult
ADD = mybir.AluOpType.add
SUB = mybir.AluOpType.subtract

D = 1024        # d_model
H = 16          # heads
DH = 64         # head dim
SQ = 1024       # query rows per core
SK = 2048       # keys per core
NKD = D // 128  # contraction chunks over d_model
NKC = SK // 128  # key chunks
NM = H * DH // 128  # output-row chunks of the projections (head pairs)
EPS = 1e-5


def _build():
    nc = bacc.Bacc("TRN2", target_bir_lowering=False)
    with tile.TileContext(nc) as tc:
        with (
            nc.allow_low_precision(reason="f32r staging intentional; fp32 accum in PSUM"),
            tc.tile_pool(name="dram", bufs=1, space="DRAM") as dram,
            tc.tile_pool(name="persist", bufs=1) as persist,
        ):
            # ---- I/O ----
            wq_t = dram.tile([D, H * DH], F32R, kind="ExternalInput", name="wq_t", uniquify=False)
            wk_t = dram.tile([D, H * DH], F32R, kind="ExternalInput", name="wk_t", uniquify=False)
            wv_t = dram.tile([D, H * DH], F32R, kind="ExternalInput", name="wv_t", uniquify=False)
            wfc_t = dram.tile([H * DH, D], F32R, kind="ExternalInput", name="wfc_t", uniquify=False)
            xq_t = dram.tile([D, SQ], F32R, kind="ExternalInput", name="xq_t", uniquify=False)
            xk_t = dram.tile([D, SK], F32R, kind="ExternalInput", name="xk_t", uniquify=False)
            xv_t = dram.tile([D, SK], F32R, kind="ExternalInput", name="xv_t", uniquify=False)
            res = dram.tile([SQ, D], F32, kind="ExternalInput", name="res", uniquify=False)
            attn_t = dram.tile([H, NKC, 128, SQ], F32R, kind="ExternalOutput", name="attn_t", uniquify=False)
            out = dram.tile([SQ, D], F32, kind="ExternalOutput", name="out", uniquify=False)
            # scratch
            kt_s = dram.tile([NM, 128, SK], F32R, name="kt_s")
            vx_s = dram.tile([NKC, 128, H, DH + 1], F32R, name="vx_s")

            # ---- persistent SBUF ----
            qt_sb = [persist.tile([128, SQ], F32R, tag=f"qt{m}", name=f"qt{m}") for m in range(NM)]
            ctx_sb = [persist.tile([128, SQ], F32R, tag=f"ctx{m}", name=f"ctx{m}") for m in range(NM)]
            ones_f = persist.tile([128, 128], F32, tag="ones_f")
            ones_r = persist.tile([1, 128], F32R, tag="ones_r")
            ones_h = persist.tile([128, H], F32R, tag="ones_h")
            eps_sb = persist.tile([128, 1], F32, tag="eps")
            nc.vector.memset(ones_f[:], 1.0)
            nc.vector.memset(eps_sb[:], EPS)
            nc.scalar.copy(out=ones_r[:], in_=ones_f[0:1, :])
            nc.scalar.copy(out=ones_h[:], in_=ones_f[:, 0:H])

            # ================= phase 1: projections =================
            def proj(w_dram, x_dram, n_cols, on_tile_done):
                """out[m-chunk] = sum_kd w[kd,:,m].T @ x[kd,:,n]  -> callback per m."""
                with (
                    tc.tile_pool(name="p1w", bufs=1) as p1w,
                    tc.tile_pool(name="p1x", bufs=1) as p1x,
                    tc.tile_pool(name="p1ps", bufs=1, space="PSUM") as p1ps,
                    tc.tile_pool(name="p1st", bufs=2) as p1st,
                ):
                    w_sb = [p1w.tile([128, H * DH], F32R, tag=f"w{kd}", name=f"w{kd}") for kd in range(NKD)]
                    x_sb = [p1x.tile([128, n_cols], F32R, tag=f"x{kd}", name=f"x{kd}") for kd in range(NKD)]
                    for kd in range(NKD):
                        nc.sync.dma_start(out=w_sb[kd][:], in_=w_dram[kd * 128:(kd + 1) * 128, :])
                        nc.sync.dma_start(out=x_sb[kd][:], in_=x_dram[kd * 128:(kd + 1) * 128, :])
                    for m in range(NM):
                        ps = p1ps.tile([128, n_cols], F32, tag="acc", bufs=2)
                        for n in range(n_cols // 512):
                            for kd in range(NKD):
                                nc.tensor.matmul(
                                    ps[:, n * 512:(n + 1) * 512],
                                    w_sb[kd][:, m * 128:(m + 1) * 128],
                                    x_sb[kd][:, n * 512:(n + 1) * 512],
                                    start=(kd == 0), stop=(kd == NKD - 1),
                                )
                        on_tile_done(m, ps, p1st)

            def qt_done(m, ps, pool):
                nc.scalar.copy(out=qt_sb[m][:], in_=ps[:])

            def kt_done(m, ps, pool):
                st = pool.tile([128, SK], F32R, tag="ktst")
                nc.scalar.copy(out=st[:], in_=ps[:])
                nc.sync.dma_start(out=kt_s[m], in_=st[:])

            proj(wq_t, xq_t, SQ, qt_done)
            proj(wk_t, xk_t, SK, kt_done)

            # V: out rows are s-chunks (lhsT = x, rhs = w), gap layout + ones col
            with (
                tc.tile_pool(name="p1w", bufs=1) as p1w,
                tc.tile_pool(name="p1x", bufs=1) as p1x,
                tc.tile_pool(name="p1ps", bufs=1, space="PSUM") as p1ps,
                tc.tile_pool(name="p1st", bufs=2) as p1st,
            ):
                w_sb = [p1w.tile([128, H * DH], F32R, tag=f"w{kd}", name=f"w{kd}") for kd in range(NKD)]
                x_sb = [p1x.tile([128, SK], F32R, tag=f"x{kd}", name=f"xv{kd}") for kd in range(NKD)]
                for kd in range(NKD):
                    nc.sync.dma_start(out=w_sb[kd][:], in_=wv_t[kd * 128:(kd + 1) * 128, :])
                    nc.sync.dma_start(out=x_sb[kd][:], in_=xv_t[kd * 128:(kd + 1) * 128, :])
                for kc in range(NKC):
                    vst = p1st.tile([128, H, DH + 1], F32R, tag="vst")
                    nc.scalar.copy(
                        out=vst[:, :, DH:DH + 1],
                        in_=ones_h[:].rearrange("p (h o) -> p h o", o=1),
                    )
                    for n in range(H * DH // 512):
                        ps = p1ps.tile([128, 512], F32, tag="acc", bufs=3)
                        for kd in range(NKD):
                            nc.tensor.matmul(
                                ps[:],
                                x_sb[kd][:, kc * 128:(kc + 1) * 128],
                                w_sb[kd][:, n * 512:(n + 1) * 512],
                                start=(kd == 0), stop=(kd == NKD - 1),
                            )
                        nc.scalar.copy(
                            out=vst[:, n * 8:(n + 1) * 8, 0:DH],
                            in_=ps[:].rearrange("p (h e) -> p h e", e=DH),
                        )
                    nc.sync.dma_start(out=vx_s[kc], in_=vst[:])

            # ================= phase 2: attention per head =================
            with (
                tc.tile_pool(name="p2kt", bufs=2) as p2kt,
                tc.tile_pool(name="p2v", bufs=2) as p2v,
                tc.tile_pool(name="p2stage", bufs=1) as p2stage,
                tc.tile_pool(name="p2small", bufs=2) as p2small,
                tc.tile_pool(name="p2ps", bufs=1, space="PSUM") as p2ps,
            ):
                for hp in range(NM):
                    kt_pair = p2kt.tile([128, SK], F32R, tag="ktp")
                    nc.sync.dma_start(out=kt_pair[:], in_=kt_s[hp])
                    v_pair = p2v.tile([128, NKC, 2, DH + 1], F32R, tag="vp")
                    nc.sync.dma_start(
                        out=v_pair[:],
                        in_=vx_s[:, :, 2 * hp:2 * hp + 2, :].transpose([1, 0, 2, 3]),
                    )
                    for h2 in range(2):
                        h = 2 * hp + h2
                        hoff = h2 * 64
                        staging = p2stage.tile([128, NKC, SQ], F32R, tag="stage")
                        ctx_ps = p2ps.tile([65, SQ], F32, tag="ctx", bufs=1)
                        for kc in range(NKC):
                            s_ps = p2ps.tile([128, SQ], F32, tag="sps", bufs=3)
                            for n in range(SQ // 512):
                                nc.tensor.matmul(
                                    s_ps[:, n * 512:(n + 1) * 512],
                                    kt_pair[hoff:hoff + 64, kc * 128:(kc + 1) * 128],
                                    qt_sb[hp][hoff:hoff + 64, n * 512:(n + 1) * 512],
                                    start=True, stop=True,
                                )
                            nc.scalar.activation(
                                out=staging[:, kc, :], in_=s_ps[:], func=AF.Exp, scale=0.125,
                            )
                            for n in range(SQ // 512):
                                nc.tensor.matmul(
                                    ctx_ps[:, n * 512:(n + 1) * 512],
                                    v_pair[:, kc, h2, :],
                                    staging[:, kc, n * 512:(n + 1) * 512],
                                    start=(kc == 0), stop=(kc == NKC - 1),
                                )
                        rinv = p2small.tile([1, SQ], F32R, tag="rinv")
                        nc.vector.reciprocal(out=rinv[:], in_=ctx_ps[64:65, :])
                        rb_ps = p2ps.tile([128, SQ], F32, tag="sps", bufs=3)
                        for n in range(SQ // 512):
                            nc.tensor.matmul(
                                rb_ps[:, n * 512:(n + 1) * 512],
                                ones_r[:],
                                rinv[0:1, n * 512:(n + 1) * 512],
                                start=True, stop=True,
                            )
                        rb_sb = p2small.tile([128, SQ], F32R, tag="rb")
                        nc.scalar.copy(out=rb_sb[:], in_=rb_ps[:])
                        nc.vector.tensor_tensor(
                            out=ctx_sb[hp][hoff:hoff + 64, :],
                            in0=ctx_ps[0:64, :], in1=rb_sb[0:64, :], op=MULT,
                        )
                        for kc in range(NKC):
                            nc.vector.tensor_tensor(
                                out=staging[:, kc, :], in0=staging[:, kc, :],
                                in1=rb_sb[:], op=MULT,
                            )
                        for g in range(NKC // 4):
                            nc.sync.dma_start(
                                out=attn_t[h, 4 * g:4 * g + 4].transpose([1, 0, 2]),
                                in_=staging[:, 4 * g:4 * g + 4, :],
                            )

            # ================= phase 3: fc + residual + layernorm =================
            with (
                tc.tile_pool(name="p3w", bufs=1) as p3w,
                tc.tile_pool(name="p3r", bufs=1) as p3r,
                tc.tile_pool(name="p3t", bufs=2) as p3t,
                tc.tile_pool(name="p3ps", bufs=2, space="PSUM") as p3ps,
            ):
                wf_sb = [p3w.tile([128, D], F32R, tag=f"f{cc}", name=f"f{cc}") for cc in range(NKD)]
                res_sb = [p3r.tile([128, D], F32, tag=f"r{m}", name=f"r{m}") for m in range(NM)]
                for cc in range(NKD):
                    nc.sync.dma_start(out=wf_sb[cc][:], in_=wfc_t[cc * 128:(cc + 1) * 128, :])
                for m in range(NM):
                    nc.sync.dma_start(out=res_sb[m][:], in_=res[m * 128:(m + 1) * 128, :])
                for mq in range(NM):
                    fc_ps = p3ps.tile([128, D], F32, tag="fc")
                    for n in range(D // 512):
                        for cc in range(NKD):
                            nc.tensor.matmul(
                                fc_ps[:, n * 512:(n + 1) * 512],
                                ctx_sb[cc][:, mq * 128:(mq + 1) * 128],
                                wf_sb[cc][:, n * 512:(n + 1) * 512],
                                start=(cc == 0), stop=(cc == NKD - 1),
                            )
                    fcr = p3t.tile([128, D], F32, tag="fcr")
                    nc.vector.tensor_tensor(out=fcr[:], in0=fc_ps[:], in1=res_sb[mq][:], op=ADD)
                    stats = p3t.tile([128, 2, 6], F32, tag="stats")
                    nc.vector.bn_stats(out=stats[:, 0, :], in_=fcr[:, 0:512])
                    nc.vector.bn_stats(out=stats[:, 1, :], in_=fcr[:, 512:1024])
                    mv = p3t.tile([128, 2], F32, tag="mv")
                    nc.vector.bn_aggr(out=mv[:], in_=stats[:])
                    sd = p3t.tile([128, 1], F32, tag="sd")
                    nc.scalar.activation(out=sd[:], in_=mv[:, 1:2], func=AF.Sqrt, bias=eps_sb[:])
                    rstd = p3t.tile([128, 1], F32, tag="rstd")
                    nc.vector.reciprocal(out=rstd[:], in_=sd[:])
                    o_sb = p3t.tile([128, D], F32, tag="osb")
                    nc.vector.tensor_scalar(
                        out=o_sb[:], in0=fcr[:], scalar1=mv[:, 0:1], scalar2=rstd[:],
                        op0=SUB, op1=MULT,
                    )
                    nc.sync.dma_start(out=out[mq * 128:(mq + 1) * 128, :], in_=o_sb[:])

    nc.compile()
    return nc


_NC_CACHE = {}


def _get_nc():
    if "nc" not in _NC_CACHE:
        _NC_CACHE["nc"] = _build()
    return _NC_CACHE["nc"]


def _in_maps(input_q, input_k, input_v, W_Q, W_K, W_V, W_fc):
    input_q = np.asarray(input_q, dtype=np.float32)
    input_k = np.asarray(input_k, dtype=np.float32)
    input_v = np.asarray(input_v, dtype=np.float32)
    wq_t = np.ascontiguousarray(np.asarray(W_Q, np.float32).T)
    wk_t = np.ascontiguousarray(np.asarray(W_K, np.float32).T)
    wv_t = np.ascontiguousarray(np.asarray(W_V, np.float32).T)
    wfc_t = np.ascontiguousarray(np.asarray(W_fc, np.float32).T)

    in_maps = []
    for c in range(8):
        b, qh = c // 2, c % 2
        rows = slice(qh * SQ, (qh + 1) * SQ)
        in_maps.append({
            "wq_t": wq_t, "wk_t": wk_t, "wv_t": wv_t, "wfc_t": wfc_t,
            "xq_t": np.ascontiguousarray(input_q[b, rows].T),
            "xk_t": np.ascontiguousarray(input_k[b].T),
            "xv_t": np.ascontiguousarray(input_v[b].T),
            "res": np.ascontiguousarray(input_q[b, rows]),
        })
    return in_maps


def _unshard(results):
    out = np.empty((4, 2 * SQ, D), np.float32)
    attn = np.empty((4, H, 2 * SQ, 2 * SQ), np.float32)
    for c in range(8):
        r = results.results[c]
        b, qh = c // 2, c % 2
        out[b, qh * SQ:(qh + 1) * SQ] = r["out"]
        at = r["attn_t"].reshape(H, SK, SQ)  # [h, k, q]
        attn[b, :, qh * SQ:(qh + 1) * SQ, :] = at.transpose(0, 2, 1)
    return out, attn


def kernel(input_q, input_k, input_v, W_Q, W_K, W_V, W_fc):
    in_maps = _in_maps(input_q, input_k, input_v, W_Q, W_K, W_V, W_fc)
    results = run_bass_kernel_spmd(_get_nc(), in_maps, core_ids=list(range(8)))
    return _unshard(results)
